# revision 21
# baseline (speedup 1.0000x reference)
"""Bahdanau attention on 8 Trainium2 NeuronCores (Bass/Tile).

reference:
    proj_v = values @ W1 + b1             # [B, S, U]
    proj_q = (query @ W2 + b2)[:, None]   # [B, 1, U]
    score  = tanh(proj_v + proj_q) @ V + bV
    attn   = softmax(score, axis=1)       # [B, S, 1]
    ctx    = sum(attn * values, axis=1)   # [B, D]

Sharding: data-parallel over batch B=32 across 8 cores (4 batches/core);
each core holds full W1/W2/V.

Device-side strategy: the big matmul (values @ W1) contracts over d,
which must live on SBUF partitions for the PE.  values arrives from HBM
in natural [s, d] layout, so the host ships a second, pre-transposed
copy valuesT [d, s] (pure layout prep, same bytes) and the kernel never
transposes on-chip:
  - scores:  psum[u,s] += W1[dchunk,uchunk].T @ valuesT[dchunk, stile]
             (float32r = fp32 bits at full PE rate), tanh+bias fused in
             one ScalarE activation (bias = (b1+b2+query@W2)[u] is
             per-partition in this orientation), then the score row via
             a PE matvec with V.  bV is dropped: softmax is
             shift-invariant.
  - softmax: flash-style without max subtraction (scores for this
             model/data are O(+-3); exp cannot overflow fp32): exp+sum
             fused in one activation(accum_out=...) per s-tile, ctx
             accumulated with UNNORMALIZED weights, one 1/Z scale at
             batch end for both outputs.
  - context: ctx[1,d] += p_col[schunk].T @ values[schunk, d] with
             natural-layout tiles; the exp row is bounced through DRAM
             to scatter it across partitions (partition-major mapping so
             the scatter reads 16B-contiguous per partition).  Context
             matmuls for s-tile i are emitted after the score matmuls of
             s-tile i+1 so the PE never waits on the exp/scatter chain.

Startup: W2 is loaded as per-uchunk column tiles and the tiny proj_q
matmul groups are interleaved into the first s-tile's j-loop; dummy
matmuls on a zeroed tile warm the PE HAM clock gate while W1/xt stream
in.
"""

import numpy as np

B, S, D, U = 32, 2048, 1024, 1024
NC = 8
NB = B // NC          # batches per core
P = 128
KC = D // P           # contraction chunks
UC = U // P           # units chunks
NST = 4               # score s-tiles per batch
ST = S // NST         # 512
TPT = ST // P         # context s-blocks per s-tile (4)
WARMUP_MMS = 16

_CACHE = {}


def _build():
    from contextlib import ExitStack

    import concourse.bacc as bacc
    import concourse.tile as tile
    from concourse import mybir

    f32 = mybir.dt.float32
    f32r = mybir.dt.float32r
    AF = mybir.ActivationFunctionType
    AX = mybir.AxisListType

    nc = bacc.Bacc("TRN2", target_bir_lowering=False, debug=False, num_devices=NC)

    xt = nc.declare_dram_parameter("xt", [NB, D, S], f32, isOutput=False)
    val = nc.declare_dram_parameter("val", [NB, S, D], f32, isOutput=False)
    qT = nc.declare_dram_parameter("qT", [D, NB], f32, isOutput=False)
    w1 = nc.declare_dram_parameter("w1", [D, U], f32, isOutput=False)
    w2 = nc.declare_dram_parameter("w2", [D, U], f32, isOutput=False)
    bc = nc.declare_dram_parameter("bc", [U, 1], f32, isOutput=False)
    vv = nc.declare_dram_parameter("vv", [U, 1], f32, isOutput=False)
    octx = nc.declare_dram_parameter("octx", [NB, D], f32, isOutput=True)
    oattn = nc.declare_dram_parameter("oattn", [NB, S], f32, isOutput=True)

    with tile.TileContext(nc) as tc, ExitStack() as ctx:
        consts = ctx.enter_context(tc.tile_pool(name="consts", bufs=1))
        xtp = ctx.enter_context(tc.tile_pool(name="xtp", bufs=3))
        ttp = ctx.enter_context(tc.tile_pool(name="ttp", bufs=4))
        nvp = ctx.enter_context(tc.tile_pool(name="nvp", bufs=2))
        rowp = ctx.enter_context(tc.tile_pool(name="rowp", bufs=3))
        smallp = ctx.enter_context(tc.tile_pool(name="smallp", bufs=3))
        pp = ctx.enter_context(tc.tile_pool(name="pp", bufs=3, space="PSUM"))
        sppp = ctx.enter_context(tc.tile_pool(name="sppp", bufs=2, space="PSUM"))
        ctxp = ctx.enter_context(tc.tile_pool(name="ctxp", bufs=3, space="PSUM"))
        dramp = ctx.enter_context(tc.tile_pool(name="dramp", bufs=3, space="DRAM"))

        # ---- prologue DMAs, batched wide (one dma_start spreads across all
        # 16 SDMA engines; >=1MiB hits ~78%+ of peak) and ordered by when the
        # PE needs the bytes ----
        qt_all = consts.tile([P, KC * NB], f32r, tag="qt", name="qt_all")
        nc.sync.dma_start(
            out=qt_all[:].rearrange("p (k b) -> p k b", k=KC),
            in_=qT[:].rearrange("(k p) b -> p k b", p=P).bitcast(f32r),
        )
        bc_all = consts.tile([P, KC], f32, tag="bc", name="bc_all")
        nc.sync.dma_start(
            out=bc_all[:].rearrange("p (k a) -> p k a", k=KC),
            in_=bc[:].rearrange("(k p) a -> p k a", p=P),
        )
        v_all = consts.tile([P, KC], f32r, tag="v", name="v_all")
        nc.sync.dma_start(
            out=v_all[:].rearrange("p (k a) -> p k a", k=KC),
            in_=vv[:].rearrange("(k p) a -> p k a", p=P).bitcast(f32r),
        )
        qt_sb = [qt_all[:, k * NB:(k + 1) * NB] for k in range(KC)]
        bc_sb = [bc_all[:, k:k + 1] for k in range(KC)]
        v_sb = [v_all[:, k:k + 1] for k in range(KC)]

        UH = U // 2

        def load_half(dst_handle, h, tag, name):
            t = consts.tile([P, KC * UH], f32r, tag=tag, name=name)
            nc.sync.dma_start(
                out=t[:].rearrange("p (k u) -> p k u", k=KC),
                in_=dst_handle[:, h * UH:(h + 1) * UH]
                .rearrange("(k p) u -> p k u", p=P)
                .bitcast(f32r),
            )
            return t

        UQ = U // 4

        def load_quarter(src_handle, q, tag, name):
            t = consts.tile([P, KC * UQ], f32r, tag=tag, name=name)
            nc.sync.dma_start(
                out=t[:].rearrange("p (k u) -> p k u", k=KC),
                in_=src_handle[:, q * UQ:(q + 1) * UQ]
                .rearrange("(k p) u -> p k u", p=P)
                .bitcast(f32r),
            )
            return t

        # DMA order = the order the PE consumes the bytes:
        #   w2h0 (proj_q j0-3) -> w1q0/q1+xt00 (scores j0-3) -> w1q2/q3
        #   (j4-7, staggered) -> w2h1 (proj_q j4-7) -> steady state
        w2h = [None, None]
        w1q = [None] * 4
        w2h[0] = load_half(w2, 0, "w2_0", "w2h0")
        w1q[0] = load_quarter(w1, 0, "w1_0", "w1q0")
        w1q[1] = load_quarter(w1, 1, "w1_1", "w1q1")
        first_xts = xtp.tile([P, KC * ST], f32r, tag="xt", name="xt0_0")
        nc.sync.dma_start(
            out=first_xts[:].rearrange("p (k s) -> p k s", k=KC),
            in_=xt[0, :, 0:ST]
            .rearrange("(k p) s -> p k s", p=P)
            .bitcast(f32r),
        )
        w2h[1] = load_half(w2, 1, "w2_1", "w2h1")
        w1q[2] = load_quarter(w1, 2, "w1_2", "w1q2")
        w1q[3] = load_quarter(w1, 3, "w1_3", "w1q3")

        def w1_lhsT(k, j):
            q, jj = divmod(j, UC // 4)
            return w1q[q][:, k * UQ + jj * P:k * UQ + (jj + 1) * P]

        bias_sb = [None] * UC

        def proj_q(j):
            qp = pp.tile([P, NB], f32, tag="proj", name=f"qp{j}")
            h, jj = divmod(j, UC // 2)
            for k in range(KC):
                nc.tensor.matmul(
                    qp[:],
                    w2h[h][:, k * UH + jj * P:k * UH + (jj + 1) * P],
                    qt_sb[k],
                    start=(k == 0), stop=(k == KC - 1),
                )
            bt = consts.tile([P, NB], f32, tag=f"bias_{j}", name=f"bias{j}")
            nc.vector.tensor_scalar_add(out=bt[:], in0=qp[:], scalar1=bc_sb[j])
            bias_sb[j] = bt

        # Warm the PE HAM clock gate while W2/W1/xt stream from HBM.  Plain
        # fp32 matmuls run 4 cycles/row, so a few fill the warmup window.
        zt = consts.tile([P, ST], f32, tag="zt", name="zt")
        nc.vector.memset(zt[:], 0.0)
        dps = pp.tile([P, ST], f32, tag="proj", name="dps")
        for i in range(WARMUP_MMS):
            nc.tensor.matmul(dps[:], zt[:, 0:P], zt[:], start=True, stop=True)

        for j in range(UC // 2):
            proj_q(j)

        # ---- per-(batch, s-tile) stages ----
        state = {}  # per-batch: pr row, zp, cps accumulators

        def batch_state(b):
            if b not in state:
                pr = rowp.tile([1, S], f32, tag="prow", name=f"pr{b}")
                zp = smallp.tile([1, 2 * NST], f32, tag="zp", name=f"zp{b}")
                nc.vector.memset(zp[:], 0.0)
                cps = [
                    ctxp.tile([1, ST], f32, tag="ctx", name=f"cp{b}_{dn}")
                    for dn in range(2)
                ]
                state[b] = (pr, zp, cps)
            return state[b]

        def score_stile(b, st, xts=None, pre_j=None):
            """64 proj matmuls + 8 tanh + 8 score matvecs for one s-tile."""
            if xts is None:
                xts = xtp.tile([P, KC * ST], f32r, tag="xt", name=f"xt{b}_{st}")
                nc.sync.dma_start(
                    out=xts[:].rearrange("p (k s) -> p k s", k=KC),
                    in_=xt[b, :, st * ST:(st + 1) * ST]
                    .rearrange("(k p) s -> p k s", p=P)
                    .bitcast(f32r),
                )
            spp = sppp.tile([1, ST], f32, tag="spp", name=f"spp{b}_{st}")
            tts = [None] * UC

            def matvec(j):
                nc.tensor.matmul(
                    spp[:], v_sb[j], tts[j][:], start=(j == 0), stop=(j == UC - 1)
                )

            for j in range(UC):
                if pre_j is not None:
                    pre_j(j)
                pj = pp.tile([P, ST], f32, tag="proj", name=f"pj{b}_{st}_{j}")
                for k in range(KC):
                    nc.tensor.matmul(
                        pj[:],
                        w1_lhsT(k, j),
                        xts[:, k * ST:(k + 1) * ST],
                        start=(k == 0),
                        stop=(k == KC - 1),
                    )
                tts[j] = ttp.tile([P, ST], f32r, tag="tt", name=f"tt{b}_{st}_{j}")
                nc.scalar.activation(
                    tts[j][:], pj[:], AF.Tanh, bias=bias_sb[j][:, b:b + 1]
                )
                if j >= 2:
                    matvec(j - 2)
            matvec(UC - 2)
            matvec(UC - 1)
            return spp

        def exp_scatter(b, st, spp, lo=0, hi=ST):
            """exp (+partial sum) of score columns [lo, hi); scatter so that
            pcol[p, t] = piece[p*tp + t] (partition-major within the piece)."""
            pr, zp, _ = batch_state(b)
            n = hi - lo
            tp = n // P
            zslot = st if lo == 0 else NST + st % NST
            nc.scalar.activation(
                pr[:, st * ST + lo:st * ST + hi],
                spp[:] if (lo == 0 and hi == ST) else spp[:, 0:n],
                AF.Exp,
                accum_out=zp[:, zslot:zslot + 1],
            )
            pbt = dramp.tile([1, n], f32, tag="pb", name=f"pb{b}_{st}_{lo}")
            nc.gpsimd.dma_start(out=pbt[:], in_=pr[:, st * ST + lo:st * ST + hi])
            pcol = smallp.tile([P, tp], f32r, tag="pcol", name=f"pc{b}_{st}_{lo}")
            nc.gpsimd.dma_start(
                out=pcol[:],
                in_=pbt[:].rearrange("a (p t) -> p (a t)", p=P).bitcast(f32r),
            )
            return pcol

        def ctx_mms(b, st, pcol, tp=TPT, piece_lo=None, first=False, last=False):
            """context matmuls (unnormalized weights) for one s-tile or piece."""
            _, _, cps = batch_state(b)
            if piece_lo is None:
                # rows follow the whole-tile mapping s = st*ST + p*TPT + t
                nv = nvp.tile([P, TPT * D], f32r, tag="nv", name=f"nv{b}_{st}")
                nc.sync.dma_start(
                    out=nv[:],
                    in_=val[b]
                    .rearrange("(g p t) d -> g p (t d)", p=P, t=TPT)[st]
                    .bitcast(f32r),
                )
            else:
                # piece mapping: rows s = st*ST + piece_lo + p*tp + t
                nv = nvp.tile([P, tp * D], f32r, tag="nv", name=f"nv{b}_{st}_{piece_lo}")
                nc.sync.dma_start(
                    out=nv[:],
                    in_=val[b, st * ST + piece_lo:st * ST + piece_lo + tp * P, :]
                    .rearrange("(p t) d -> p (t d)", p=P)
                    .bitcast(f32r),
                )
            for tloc in range(tp):
                for dn in range(2):
                    nc.tensor.matmul(
                        cps[dn][:],
                        pcol[:, tloc:tloc + 1],
                        nv[:, tloc * D + dn * ST:tloc * D + (dn + 1) * ST],
                        start=(first and tloc == 0),
                        stop=(last and tloc == tp - 1),
                    )

        def finalize(b):
            """1/Z normalization of both outputs; DMA out."""
            pr, zp, cps = batch_state(b)
            z = smallp.tile([1, 1], f32, tag="z", name=f"z{b}")
            nc.vector.reduce_sum(out=z[:], in_=zp[:], axis=AX.X)
            rz = smallp.tile([1, 1], f32, tag="rz", name=f"rz{b}")
            nc.vector.reciprocal(rz[:], z[:])
            at = rowp.tile([1, S], f32, tag="prow", name=f"at{b}")
            nc.vector.tensor_scalar_mul(out=at[:], in0=pr[:], scalar1=rz[:, 0:1])
            nc.gpsimd.dma_start(out=oattn[b:b + 1, :], in_=at[:])
            crow = smallp.tile([1, D], f32, tag="crow", name=f"cr{b}")
            for dn in range(2):
                nc.vector.tensor_scalar_mul(
                    out=crow[:, dn * ST:(dn + 1) * ST], in0=cps[dn][:], scalar1=rz[:, 0:1]
                )
            nc.gpsimd.dma_start(out=octx[b:b + 1, :], in_=crow[:])
            del state[b]

        # s-tile software pipeline: ctx matmuls of tile i run after the score
        # matmuls of tile i+1, so the PE never waits on exp/scatter.
        tasks = [(b, st) for b in range(NB) for st in range(NST)]
        pend = None
        for idx, (b, st) in enumerate(tasks):
            last = idx == len(tasks) - 1
            if b == 0 and st == 0:
                spp = score_stile(
                    b, st, xts=first_xts,
                    pre_j=lambda j: (proj_q(j) if j >= UC // 2 else None),
                )
            else:
                spp = score_stile(b, st)
            if last:
                # emit the final exp/scatter before pend's nv DMAs so the
                # tail-critical scatter isn't queued behind bulk traffic
                mine = exp_scatter(b, st, spp)
            if pend is not None:
                pb_, pst_, pcol_ = pend
                ctx_mms(pb_, pst_, pcol_, first=(pst_ == 0),
                        last=(pst_ == NST - 1))
                if pst_ == NST - 1:
                    finalize(pb_)
            pend = (b, st, exp_scatter(b, st, spp)) if not last else (b, st, mine)
        pb_, pst_, pcol_ = pend
        # keep the PE busy (and the HAM clock warm) while the final
        # exp/scatter chain completes
        tdps = pp.tile([P, ST], f32, tag="proj", name="tdps")
        for i in range(12):
            nc.tensor.matmul(tdps[:], zt[:, 0:P], zt[:], start=True, stop=True)
        ctx_mms(pb_, pst_, pcol_, first=(pst_ == 0), last=(pst_ == NST - 1))
        finalize(pb_)

    nc.compile()
    return nc


def kernel(query, values, W1, b1, W2, b2, V, bV, _trace=False, _trace_kwargs=None):
    from concourse.bass_utils import run_bass_kernel_spmd

    query = np.asarray(query, dtype=np.float32)
    values = np.asarray(values, dtype=np.float32)
    W1 = np.asarray(W1, dtype=np.float32)
    b1 = np.asarray(b1, dtype=np.float32)
    W2 = np.asarray(W2, dtype=np.float32)
    b2 = np.asarray(b2, dtype=np.float32)
    V = np.asarray(V, dtype=np.float32)

    assert query.shape == (B, D) and values.shape == (B, S, D)

    if "nc" not in _CACHE:
        _CACHE["nc"] = _build()
    nc = _CACHE["nc"]

    valuesT = np.ascontiguousarray(values.transpose(0, 2, 1))  # [B, D, S]
    qTf = np.ascontiguousarray(query.T)                        # [D, B]
    bcf = np.ascontiguousarray((b1 + b2).reshape(U, 1))
    Vf = np.ascontiguousarray(V.reshape(U, 1))

    in_maps = []
    for c in range(NC):
        lo, hi = c * NB, (c + 1) * NB
        in_maps.append({
            "xt": valuesT[lo:hi],
            "val": values[lo:hi],
            "qT": np.ascontiguousarray(qTf[:, lo:hi]),
            "w1": W1,
            "w2": W2,
            "bc": bcf,
            "vv": Vf,
        })

    res = run_bass_kernel_spmd(
        nc, in_maps, list(range(NC)), trace=_trace, **(_trace_kwargs or {})
    )
    _CACHE["last_result"] = res

    context = np.concatenate([res.results[c]["octx"] for c in range(NC)], axis=0)
    attn = np.concatenate([res.results[c]["oattn"] for c in range(NC)], axis=0)
    return context, attn.reshape(B, S, 1)


# revision 22
# speedup vs baseline: 1.0070x; 1.0070x over previous
"""Bahdanau attention on 8 Trainium2 NeuronCores (Bass/Tile).

reference:
    proj_v = values @ W1 + b1             # [B, S, U]
    proj_q = (query @ W2 + b2)[:, None]   # [B, 1, U]
    score  = tanh(proj_v + proj_q) @ V + bV
    attn   = softmax(score, axis=1)       # [B, S, 1]
    ctx    = sum(attn * values, axis=1)   # [B, D]

Sharding: data-parallel over batch B=32 across 8 cores (4 batches/core);
each core holds full W1/W2/V.

Device-side strategy: the big matmul (values @ W1) contracts over d,
which must live on SBUF partitions for the PE.  values arrives from HBM
in natural [s, d] layout, so the host ships a second, pre-transposed
copy valuesT [d, s] (pure layout prep, same bytes) and the kernel never
transposes on-chip:
  - scores:  psum[u,s] += W1[dchunk,uchunk].T @ valuesT[dchunk, stile]
             (float32r = fp32 bits at full PE rate), tanh+bias fused in
             one ScalarE activation (bias = (b1+b2+query@W2)[u] is
             per-partition in this orientation), then the score row via
             a PE matvec with V.  bV is dropped: softmax is
             shift-invariant.
  - softmax: flash-style without max subtraction (scores for this
             model/data are O(+-3); exp cannot overflow fp32): exp+sum
             fused in one activation(accum_out=...) per s-tile, ctx
             accumulated with UNNORMALIZED weights, one 1/Z scale at
             batch end for both outputs.
  - context: ctx[1,d] += p_col[schunk].T @ values[schunk, d] with
             natural-layout tiles; the exp row is bounced through DRAM
             to scatter it across partitions (partition-major mapping so
             the scatter reads 16B-contiguous per partition).  Context
             matmuls for s-tile i are emitted after the score matmuls of
             s-tile i+1 so the PE never waits on the exp/scatter chain.

Startup: W2 is loaded as per-uchunk column tiles and the tiny proj_q
matmul groups are interleaved into the first s-tile's j-loop; dummy
matmuls on a zeroed tile warm the PE HAM clock gate while W1/xt stream
in.
"""

import numpy as np

B, S, D, U = 32, 2048, 1024, 1024
NC = 8
NB = B // NC          # batches per core
P = 128
KC = D // P           # contraction chunks
UC = U // P           # units chunks
NST = 4               # score s-tiles per batch
ST = S // NST         # 512
TPT = ST // P         # context s-blocks per s-tile (4)
WARMUP_MMS = 16

_CACHE = {}


def _build():
    from contextlib import ExitStack

    import concourse.bacc as bacc
    import concourse.tile as tile
    from concourse import mybir

    f32 = mybir.dt.float32
    f32r = mybir.dt.float32r
    AF = mybir.ActivationFunctionType
    AX = mybir.AxisListType

    nc = bacc.Bacc("TRN2", target_bir_lowering=False, debug=False, num_devices=NC)

    xt = nc.declare_dram_parameter("xt", [NB, D, S], f32, isOutput=False)
    val = nc.declare_dram_parameter("val", [NB, S, D], f32, isOutput=False)
    qT = nc.declare_dram_parameter("qT", [D, NB], f32, isOutput=False)
    w1 = nc.declare_dram_parameter("w1", [D, U], f32, isOutput=False)
    w2 = nc.declare_dram_parameter("w2", [D, U], f32, isOutput=False)
    bc = nc.declare_dram_parameter("bc", [U, 1], f32, isOutput=False)
    vv = nc.declare_dram_parameter("vv", [U, 1], f32, isOutput=False)
    octx = nc.declare_dram_parameter("octx", [NB, D], f32, isOutput=True)
    oattn = nc.declare_dram_parameter("oattn", [NB, S], f32, isOutput=True)

    with tile.TileContext(nc) as tc, ExitStack() as ctx:
        consts = ctx.enter_context(tc.tile_pool(name="consts", bufs=1))
        xtp = ctx.enter_context(tc.tile_pool(name="xtp", bufs=3))
        ttp = ctx.enter_context(tc.tile_pool(name="ttp", bufs=4))
        nvp = ctx.enter_context(tc.tile_pool(name="nvp", bufs=2))
        rowp = ctx.enter_context(tc.tile_pool(name="rowp", bufs=3))
        smallp = ctx.enter_context(tc.tile_pool(name="smallp", bufs=3))
        pp = ctx.enter_context(tc.tile_pool(name="pp", bufs=3, space="PSUM"))
        sppp = ctx.enter_context(tc.tile_pool(name="sppp", bufs=2, space="PSUM"))
        ctxp = ctx.enter_context(tc.tile_pool(name="ctxp", bufs=3, space="PSUM"))
        dramp = ctx.enter_context(tc.tile_pool(name="dramp", bufs=3, space="DRAM"))

        # ---- prologue DMAs, batched wide (one dma_start spreads across all
        # 16 SDMA engines; >=1MiB hits ~78%+ of peak) and ordered by when the
        # PE needs the bytes ----
        qt_all = consts.tile([P, KC * NB], f32r, tag="qt", name="qt_all")
        nc.sync.dma_start(
            out=qt_all[:].rearrange("p (k b) -> p k b", k=KC),
            in_=qT[:].rearrange("(k p) b -> p k b", p=P).bitcast(f32r),
        )
        bc_all = consts.tile([P, KC], f32, tag="bc", name="bc_all")
        nc.sync.dma_start(
            out=bc_all[:].rearrange("p (k a) -> p k a", k=KC),
            in_=bc[:].rearrange("(k p) a -> p k a", p=P),
        )
        v_all = consts.tile([P, KC], f32r, tag="v", name="v_all")
        nc.sync.dma_start(
            out=v_all[:].rearrange("p (k a) -> p k a", k=KC),
            in_=vv[:].rearrange("(k p) a -> p k a", p=P).bitcast(f32r),
        )
        qt_sb = [qt_all[:, k * NB:(k + 1) * NB] for k in range(KC)]
        bc_sb = [bc_all[:, k:k + 1] for k in range(KC)]
        v_sb = [v_all[:, k:k + 1] for k in range(KC)]

        UH = U // 2

        def load_half(dst_handle, h, tag, name):
            t = consts.tile([P, KC * UH], f32r, tag=tag, name=name)
            nc.sync.dma_start(
                out=t[:].rearrange("p (k u) -> p k u", k=KC),
                in_=dst_handle[:, h * UH:(h + 1) * UH]
                .rearrange("(k p) u -> p k u", p=P)
                .bitcast(f32r),
            )
            return t

        UQ = U // 4

        def load_quarter(src_handle, q, tag, name):
            t = consts.tile([P, KC * UQ], f32r, tag=tag, name=name)
            nc.sync.dma_start(
                out=t[:].rearrange("p (k u) -> p k u", k=KC),
                in_=src_handle[:, q * UQ:(q + 1) * UQ]
                .rearrange("(k p) u -> p k u", p=P)
                .bitcast(f32r),
            )
            return t

        # DMA order = the order the PE consumes the bytes:
        #   w2h0 (proj_q j0-3) -> w1q0/q1+xt00 (scores j0-3) -> w1q2/q3
        #   (j4-7, staggered) -> w2h1 (proj_q j4-7) -> steady state
        w2h = [None, None]
        w1q = [None] * 4
        w2h[0] = load_half(w2, 0, "w2_0", "w2h0")
        w1q[0] = load_quarter(w1, 0, "w1_0", "w1q0")
        w1q[1] = load_quarter(w1, 1, "w1_1", "w1q1")
        first_xts = xtp.tile([P, KC * ST], f32r, tag="xt", name="xt0_0")
        nc.sync.dma_start(
            out=first_xts[:].rearrange("p (k s) -> p k s", k=KC),
            in_=xt[0, :, 0:ST]
            .rearrange("(k p) s -> p k s", p=P)
            .bitcast(f32r),
        )
        w1q[2] = load_quarter(w1, 2, "w1_2", "w1q2")
        w1q[3] = load_quarter(w1, 3, "w1_3", "w1q3")
        w2h[1] = load_half(w2, 1, "w2_1", "w2h1")

        def w1_lhsT(k, j):
            q, jj = divmod(j, UC // 4)
            return w1q[q][:, k * UQ + jj * P:k * UQ + (jj + 1) * P]

        bias_sb = [None] * UC

        def proj_q(j):
            qp = pp.tile([P, NB], f32, tag="proj", name=f"qp{j}")
            h, jj = divmod(j, UC // 2)
            for k in range(KC):
                nc.tensor.matmul(
                    qp[:],
                    w2h[h][:, k * UH + jj * P:k * UH + (jj + 1) * P],
                    qt_sb[k],
                    start=(k == 0), stop=(k == KC - 1),
                )
            bt = consts.tile([P, NB], f32, tag=f"bias_{j}", name=f"bias{j}")
            nc.vector.tensor_scalar_add(out=bt[:], in0=qp[:], scalar1=bc_sb[j])
            bias_sb[j] = bt

        # Warm the PE HAM clock gate while W2/W1/xt stream from HBM.  Plain
        # fp32 matmuls run 4 cycles/row, so a few fill the warmup window.
        zt = consts.tile([P, ST], f32, tag="zt", name="zt")
        nc.vector.memset(zt[:], 0.0)
        dps = pp.tile([P, ST], f32, tag="proj", name="dps")
        for i in range(WARMUP_MMS):
            nc.tensor.matmul(dps[:], zt[:, 0:P], zt[:], start=True, stop=True)

        for j in range(UC // 2):
            proj_q(j)

        # ---- per-(batch, s-tile) stages ----
        state = {}  # per-batch: pr row, zp, cps accumulators

        def batch_state(b):
            if b not in state:
                pr = rowp.tile([1, S], f32, tag="prow", name=f"pr{b}")
                zp = smallp.tile([1, 2 * NST], f32, tag="zp", name=f"zp{b}")
                nc.vector.memset(zp[:], 0.0)
                cps = [
                    ctxp.tile([1, ST], f32, tag="ctx", name=f"cp{b}_{dn}")
                    for dn in range(2)
                ]
                state[b] = (pr, zp, cps)
            return state[b]

        def score_stile(b, st, xts=None, pre_j=None):
            """64 proj matmuls + 8 tanh + 8 score matvecs for one s-tile."""
            if xts is None:
                xts = xtp.tile([P, KC * ST], f32r, tag="xt", name=f"xt{b}_{st}")
                nc.sync.dma_start(
                    out=xts[:].rearrange("p (k s) -> p k s", k=KC),
                    in_=xt[b, :, st * ST:(st + 1) * ST]
                    .rearrange("(k p) s -> p k s", p=P)
                    .bitcast(f32r),
                )
            spp = sppp.tile([1, ST], f32, tag="spp", name=f"spp{b}_{st}")
            tts = [None] * UC

            def matvec(j):
                nc.tensor.matmul(
                    spp[:], v_sb[j], tts[j][:], start=(j == 0), stop=(j == UC - 1)
                )

            for j in range(UC):
                if pre_j is not None:
                    pre_j(j)
                pj = pp.tile([P, ST], f32, tag="proj", name=f"pj{b}_{st}_{j}")
                for k in range(KC):
                    nc.tensor.matmul(
                        pj[:],
                        w1_lhsT(k, j),
                        xts[:, k * ST:(k + 1) * ST],
                        start=(k == 0),
                        stop=(k == KC - 1),
                    )
                tts[j] = ttp.tile([P, ST], f32r, tag="tt", name=f"tt{b}_{st}_{j}")
                nc.scalar.activation(
                    tts[j][:], pj[:], AF.Tanh, bias=bias_sb[j][:, b:b + 1]
                )
                if j >= 2:
                    matvec(j - 2)
            matvec(UC - 2)
            matvec(UC - 1)
            return spp

        def exp_scatter(b, st, spp, lo=0, hi=ST):
            """exp (+partial sum) of score columns [lo, hi); scatter so that
            pcol[p, t] = piece[p*tp + t] (partition-major within the piece)."""
            pr, zp, _ = batch_state(b)
            n = hi - lo
            tp = n // P
            zslot = st if lo == 0 else NST + st % NST
            nc.scalar.activation(
                pr[:, st * ST + lo:st * ST + hi],
                spp[:] if (lo == 0 and hi == ST) else spp[:, 0:n],
                AF.Exp,
                accum_out=zp[:, zslot:zslot + 1],
            )
            pbt = dramp.tile([1, n], f32, tag="pb", name=f"pb{b}_{st}_{lo}")
            nc.gpsimd.dma_start(out=pbt[:], in_=pr[:, st * ST + lo:st * ST + hi])
            pcol = smallp.tile([P, tp], f32r, tag="pcol", name=f"pc{b}_{st}_{lo}")
            nc.gpsimd.dma_start(
                out=pcol[:],
                in_=pbt[:].rearrange("a (p t) -> p (a t)", p=P).bitcast(f32r),
            )
            return pcol

        def ctx_mms(b, st, pcol, tp=TPT, piece_lo=None, first=False, last=False):
            """context matmuls (unnormalized weights) for one s-tile or piece."""
            _, _, cps = batch_state(b)
            if piece_lo is None:
                # rows follow the whole-tile mapping s = st*ST + p*TPT + t
                nv = nvp.tile([P, TPT * D], f32r, tag="nv", name=f"nv{b}_{st}")
                nc.sync.dma_start(
                    out=nv[:],
                    in_=val[b]
                    .rearrange("(g p t) d -> g p (t d)", p=P, t=TPT)[st]
                    .bitcast(f32r),
                )
            else:
                # piece mapping: rows s = st*ST + piece_lo + p*tp + t
                nv = nvp.tile([P, tp * D], f32r, tag="nv", name=f"nv{b}_{st}_{piece_lo}")
                nc.sync.dma_start(
                    out=nv[:],
                    in_=val[b, st * ST + piece_lo:st * ST + piece_lo + tp * P, :]
                    .rearrange("(p t) d -> p (t d)", p=P)
                    .bitcast(f32r),
                )
            for tloc in range(tp):
                for dn in range(2):
                    nc.tensor.matmul(
                        cps[dn][:],
                        pcol[:, tloc:tloc + 1],
                        nv[:, tloc * D + dn * ST:tloc * D + (dn + 1) * ST],
                        start=(first and tloc == 0),
                        stop=(last and tloc == tp - 1),
                    )

        def finalize(b):
            """1/Z normalization of both outputs; DMA out."""
            pr, zp, cps = batch_state(b)
            z = smallp.tile([1, 1], f32, tag="z", name=f"z{b}")
            nc.vector.reduce_sum(out=z[:], in_=zp[:], axis=AX.X)
            rz = smallp.tile([1, 1], f32, tag="rz", name=f"rz{b}")
            nc.vector.reciprocal(rz[:], z[:])
            at = rowp.tile([1, S], f32, tag="prow", name=f"at{b}")
            nc.vector.tensor_scalar_mul(out=at[:], in0=pr[:], scalar1=rz[:, 0:1])
            nc.gpsimd.dma_start(out=oattn[b:b + 1, :], in_=at[:])
            crow = smallp.tile([1, D], f32, tag="crow", name=f"cr{b}")
            for dn in range(2):
                nc.vector.tensor_scalar_mul(
                    out=crow[:, dn * ST:(dn + 1) * ST], in0=cps[dn][:], scalar1=rz[:, 0:1]
                )
            nc.gpsimd.dma_start(out=octx[b:b + 1, :], in_=crow[:])
            del state[b]

        # s-tile software pipeline: ctx matmuls of tile i run after the score
        # matmuls of tile i+1, so the PE never waits on exp/scatter.
        tasks = [(b, st) for b in range(NB) for st in range(NST)]
        pend = None
        for idx, (b, st) in enumerate(tasks):
            last = idx == len(tasks) - 1
            if b == 0 and st == 0:
                spp = score_stile(
                    b, st, xts=first_xts,
                    pre_j=lambda j: (proj_q(j) if j >= UC // 2 else None),
                )
            else:
                spp = score_stile(b, st)
            if last:
                # emit the final exp/scatter before pend's nv DMAs so the
                # tail-critical scatter isn't queued behind bulk traffic
                mine = exp_scatter(b, st, spp)
            if pend is not None:
                pb_, pst_, pcol_ = pend
                ctx_mms(pb_, pst_, pcol_, first=(pst_ == 0),
                        last=(pst_ == NST - 1))
                if pst_ == NST - 1:
                    finalize(pb_)
            pend = (b, st, exp_scatter(b, st, spp)) if not last else (b, st, mine)
        pb_, pst_, pcol_ = pend
        # keep the PE busy (and the HAM clock warm) while the final
        # exp/scatter chain completes
        tdps = pp.tile([P, ST], f32, tag="proj", name="tdps")
        for i in range(12):
            nc.tensor.matmul(tdps[:], zt[:, 0:P], zt[:], start=True, stop=True)
        ctx_mms(pb_, pst_, pcol_, first=(pst_ == 0), last=(pst_ == NST - 1))
        finalize(pb_)

    nc.compile()
    return nc


def kernel(query, values, W1, b1, W2, b2, V, bV, _trace=False, _trace_kwargs=None):
    from concourse.bass_utils import run_bass_kernel_spmd

    query = np.asarray(query, dtype=np.float32)
    values = np.asarray(values, dtype=np.float32)
    W1 = np.asarray(W1, dtype=np.float32)
    b1 = np.asarray(b1, dtype=np.float32)
    W2 = np.asarray(W2, dtype=np.float32)
    b2 = np.asarray(b2, dtype=np.float32)
    V = np.asarray(V, dtype=np.float32)

    assert query.shape == (B, D) and values.shape == (B, S, D)

    if "nc" not in _CACHE:
        _CACHE["nc"] = _build()
    nc = _CACHE["nc"]

    valuesT = np.ascontiguousarray(values.transpose(0, 2, 1))  # [B, D, S]
    qTf = np.ascontiguousarray(query.T)                        # [D, B]
    bcf = np.ascontiguousarray((b1 + b2).reshape(U, 1))
    Vf = np.ascontiguousarray(V.reshape(U, 1))

    in_maps = []
    for c in range(NC):
        lo, hi = c * NB, (c + 1) * NB
        in_maps.append({
            "xt": valuesT[lo:hi],
            "val": values[lo:hi],
            "qT": np.ascontiguousarray(qTf[:, lo:hi]),
            "w1": W1,
            "w2": W2,
            "bc": bcf,
            "vv": Vf,
        })

    res = run_bass_kernel_spmd(
        nc, in_maps, list(range(NC)), trace=_trace, **(_trace_kwargs or {})
    )
    _CACHE["last_result"] = res

    context = np.concatenate([res.results[c]["octx"] for c in range(NC)], axis=0)
    attn = np.concatenate([res.results[c]["oattn"] for c in range(NC)], axis=0)
    return context, attn.reshape(B, S, 1)


# revision 26
# speedup vs baseline: 1.0998x; 1.0921x over previous
"""Bahdanau attention on 8 Trainium2 NeuronCores (Bass/Tile).

reference:
    proj_v = values @ W1 + b1             # [B, S, U]
    proj_q = (query @ W2 + b2)[:, None]   # [B, 1, U]
    score  = tanh(proj_v + proj_q) @ V + bV
    attn   = softmax(score, axis=1)       # [B, S, 1]
    ctx    = sum(attn * values, axis=1)   # [B, D]

Sharding: data-parallel over batch B=32 across 8 cores (4 batches/core);
each core holds full W1/W2/V.

Device-side strategy: the big matmul (values @ W1) contracts over d,
which must live on SBUF partitions for the PE.  values arrives from HBM
in natural [s, d] layout, so the host ships a second, pre-transposed
copy valuesT [d, s] (pure layout prep, same bytes) and the kernel never
transposes on-chip:
  - scores:  psum[u,s] += W1[dchunk,uchunk].T @ valuesT[dchunk, stile]
             (float32r = fp32 bits at full PE rate), tanh+bias fused in
             one ScalarE activation (bias = (b1+b2+query@W2)[u] is
             per-partition in this orientation).  The V projection runs
             on the Vector engine (per-partition multiply-accumulate)
             with a single ones-column matmul per s-tile doing the
             cross-partition reduce, keeping 7 of 8 matvec matmuls off
             the saturated PE.  bV is dropped: softmax is
             shift-invariant.
  - softmax: flash-style without max subtraction (scores for this
             model/data are O(+-3); exp cannot overflow fp32): exp+sum
             fused in one activation(accum_out=...) per s-tile, ctx
             accumulated with UNNORMALIZED weights, one 1/Z scale at
             batch end for both outputs.
  - context: ctx[1,d] += p_col[schunk].T @ values[schunk, d] with
             natural-layout tiles; the exp row is bounced through DRAM
             to scatter it across partitions (partition-major mapping so
             the scatter reads 16B-contiguous per partition).  Context
             matmuls for s-tile i are emitted after the score matmuls of
             s-tile i+1 so the PE never waits on the exp/scatter chain.

Startup: W2 is loaded as per-uchunk column tiles and the tiny proj_q
matmul groups are interleaved into the first s-tile's j-loop; dummy
matmuls on a zeroed tile warm the PE HAM clock gate while W1/xt stream
in.
"""

import numpy as np

B, S, D, U = 32, 2048, 1024, 1024
NC = 8
NB = B // NC          # batches per core
P = 128
KC = D // P           # contraction chunks
UC = U // P           # units chunks
NST = 4               # score s-tiles per batch
ST = S // NST         # 512
TPT = ST // P         # context s-blocks per s-tile (4)
WARMUP_MMS = 16

_CACHE = {}


def _build():
    from contextlib import ExitStack

    import concourse.bacc as bacc
    import concourse.tile as tile
    from concourse import mybir

    f32 = mybir.dt.float32
    f32r = mybir.dt.float32r
    AF = mybir.ActivationFunctionType
    AX = mybir.AxisListType

    nc = bacc.Bacc("TRN2", target_bir_lowering=False, debug=False, num_devices=NC)

    xt = nc.declare_dram_parameter("xt", [NB, D, S], f32, isOutput=False)
    val = nc.declare_dram_parameter("val", [NB, S, D], f32, isOutput=False)
    qT = nc.declare_dram_parameter("qT", [D, NB], f32, isOutput=False)
    w1 = nc.declare_dram_parameter("w1", [D, U], f32, isOutput=False)
    w2 = nc.declare_dram_parameter("w2", [D, U], f32, isOutput=False)
    bc = nc.declare_dram_parameter("bc", [U, 1], f32, isOutput=False)
    vv = nc.declare_dram_parameter("vv", [U, 1], f32, isOutput=False)
    one = nc.declare_dram_parameter("one", [P, 1], f32, isOutput=False)
    octx = nc.declare_dram_parameter("octx", [NB, D], f32, isOutput=True)
    oattn = nc.declare_dram_parameter("oattn", [NB, S], f32, isOutput=True)

    with tile.TileContext(nc) as tc, ExitStack() as ctx:
        consts = ctx.enter_context(tc.tile_pool(name="consts", bufs=1))
        xtp = ctx.enter_context(tc.tile_pool(name="xtp", bufs=2))
        ttp = ctx.enter_context(tc.tile_pool(name="ttp", bufs=4))
        accp = ctx.enter_context(tc.tile_pool(name="accp", bufs=2))
        nvp = ctx.enter_context(tc.tile_pool(name="nvp", bufs=2))
        rowp = ctx.enter_context(tc.tile_pool(name="rowp", bufs=3))
        smallp = ctx.enter_context(tc.tile_pool(name="smallp", bufs=3))
        pp = ctx.enter_context(tc.tile_pool(name="pp", bufs=3, space="PSUM"))
        sppp = ctx.enter_context(tc.tile_pool(name="sppp", bufs=2, space="PSUM"))
        ctxp = ctx.enter_context(tc.tile_pool(name="ctxp", bufs=3, space="PSUM"))
        dramp = ctx.enter_context(tc.tile_pool(name="dramp", bufs=3, space="DRAM"))

        # ---- prologue DMAs, batched wide (one dma_start spreads across all
        # 16 SDMA engines; >=1MiB hits ~78%+ of peak) and ordered by when the
        # PE needs the bytes ----
        qt_all = consts.tile([P, KC * NB], f32r, tag="qt", name="qt_all")
        nc.sync.dma_start(
            out=qt_all[:].rearrange("p (k b) -> p k b", k=KC),
            in_=qT[:].rearrange("(k p) b -> p k b", p=P).bitcast(f32r),
        )
        bc_all = consts.tile([P, KC], f32, tag="bc", name="bc_all")
        nc.sync.dma_start(
            out=bc_all[:].rearrange("p (k a) -> p k a", k=KC),
            in_=bc[:].rearrange("(k p) a -> p k a", p=P),
        )
        v_all = consts.tile([P, KC], f32r, tag="v", name="v_all")
        nc.sync.dma_start(
            out=v_all[:].rearrange("p (k a) -> p k a", k=KC),
            in_=vv[:].rearrange("(k p) a -> p k a", p=P).bitcast(f32r),
        )
        one_sb = consts.tile([P, 1], f32r, tag="one", name="one_sb")
        nc.sync.dma_start(out=one_sb[:], in_=one[:].bitcast(f32r))
        qt_sb = [qt_all[:, k * NB:(k + 1) * NB] for k in range(KC)]
        bc_sb = [bc_all[:, k:k + 1] for k in range(KC)]
        v_sb = [v_all[:, k:k + 1] for k in range(KC)]

        UH = U // 2

        def load_half(dst_handle, h, tag, name):
            t = consts.tile([P, KC * UH], f32r, tag=tag, name=name)
            nc.sync.dma_start(
                out=t[:].rearrange("p (k u) -> p k u", k=KC),
                in_=dst_handle[:, h * UH:(h + 1) * UH]
                .rearrange("(k p) u -> p k u", p=P)
                .bitcast(f32r),
            )
            return t

        UQ = U // 4

        def load_quarter(src_handle, q, tag, name):
            t = consts.tile([P, KC * UQ], f32r, tag=tag, name=name)
            nc.sync.dma_start(
                out=t[:].rearrange("p (k u) -> p k u", k=KC),
                in_=src_handle[:, q * UQ:(q + 1) * UQ]
                .rearrange("(k p) u -> p k u", p=P)
                .bitcast(f32r),
            )
            return t

        # DMA order = the order the PE consumes the bytes:
        #   w2h0 (proj_q j0-3) -> w1q0/q1+xt00 (scores j0-3) -> w1q2/q3
        #   (j4-7, staggered) -> w2h1 (proj_q j4-7) -> steady state
        w2h = [None, None]
        w1q = [None] * 4
        w2h[0] = load_half(w2, 0, "w2_0", "w2h0")
        w1q[0] = load_quarter(w1, 0, "w1_0", "w1q0")
        w1q[1] = load_quarter(w1, 1, "w1_1", "w1q1")
        first_xts = xtp.tile([P, KC * ST], f32r, tag="xt", name="xt0_0")
        nc.sync.dma_start(
            out=first_xts[:].rearrange("p (k s) -> p k s", k=KC),
            in_=xt[0, :, 0:ST]
            .rearrange("(k p) s -> p k s", p=P)
            .bitcast(f32r),
        )
        w1q[2] = load_quarter(w1, 2, "w1_2", "w1q2")
        w1q[3] = load_quarter(w1, 3, "w1_3", "w1q3")
        w2h[1] = load_half(w2, 1, "w2_1", "w2h1")

        def w1_lhsT(k, j):
            q, jj = divmod(j, UC // 4)
            return w1q[q][:, k * UQ + jj * P:k * UQ + (jj + 1) * P]

        bias_sb = [None] * UC

        def proj_q(j):
            qp = pp.tile([P, NB], f32, tag="proj", name=f"qp{j}")
            h, jj = divmod(j, UC // 2)
            for k in range(KC):
                nc.tensor.matmul(
                    qp[:],
                    w2h[h][:, k * UH + jj * P:k * UH + (jj + 1) * P],
                    qt_sb[k],
                    start=(k == 0), stop=(k == KC - 1),
                )
            bt = consts.tile([P, NB], f32, tag=f"bias_{j}", name=f"bias{j}")
            nc.vector.tensor_scalar_add(out=bt[:], in0=qp[:], scalar1=bc_sb[j])
            bias_sb[j] = bt

        # Warm the PE HAM clock gate while W2/W1/xt stream from HBM.  Plain
        # fp32 matmuls run 4 cycles/row, so a few fill the warmup window.
        zt = consts.tile([P, ST], f32, tag="zt", name="zt")
        nc.vector.memset(zt[:], 0.0)
        dps = pp.tile([P, ST], f32, tag="proj", name="dps")
        for i in range(WARMUP_MMS):
            nc.tensor.matmul(dps[:], zt[:, 0:P], zt[:], start=True, stop=True)

        for j in range(UC // 2):
            proj_q(j)

        # ---- per-(batch, s-tile) stages ----
        state = {}  # per-batch: pr row, zp, cps accumulators

        def batch_state(b):
            if b not in state:
                pr = rowp.tile([1, S], f32, tag="prow", name=f"pr{b}")
                zp = smallp.tile([1, 2 * NST], f32, tag="zp", name=f"zp{b}")
                nc.vector.memset(zp[:], 0.0)
                cps = [
                    ctxp.tile([1, ST], f32, tag="ctx", name=f"cp{b}_{dn}")
                    for dn in range(2)
                ]
                state[b] = (pr, zp, cps)
            return state[b]

        def score_stile(b, st, xts=None, pre_j=None):
            """64 proj matmuls + 8 tanh for one s-tile; the V matvec runs on
            DVE (per-partition multiply) + GpSimd (cross-partition reduce) to
            keep those 8 N=512 matmuls off the PE."""
            if xts is None:
                xts = xtp.tile([P, KC * ST], f32r, tag="xt", name=f"xt{b}_{st}")
                nc.sync.dma_start(
                    out=xts[:].rearrange("p (k s) -> p k s", k=KC),
                    in_=xt[b, :, st * ST:(st + 1) * ST]
                    .rearrange("(k p) s -> p k s", p=P)
                    .bitcast(f32r),
                )
            acc = accp.tile([P, ST], f32r, tag="acc", name=f"acc{b}_{st}")
            tmp = accp.tile([P, ST], f32, tag="tmp", name=f"tmp{b}_{st}")
            spp = sppp.tile([1, ST], f32, tag="spp", name=f"spp{b}_{st}")
            tts = [None] * UC

            def matvec(j):
                vj = v_all[:, j:j + 1].bitcast(f32)
                if j == 0:
                    nc.vector.tensor_scalar_mul(
                        out=acc[:], in0=tts[j][:].bitcast(f32), scalar1=vj
                    )
                else:
                    nc.vector.tensor_scalar_mul(
                        out=tmp[:], in0=tts[j][:].bitcast(f32), scalar1=vj
                    )
                    nc.vector.tensor_tensor(
                        out=acc[:], in0=acc[:].bitcast(f32), in1=tmp[:],
                        op=mybir.AluOpType.add,
                    )

            for j in range(UC):
                if pre_j is not None:
                    pre_j(j)
                pj = pp.tile([P, ST], f32, tag="proj", name=f"pj{b}_{st}_{j}")
                for k in range(KC):
                    nc.tensor.matmul(
                        pj[:],
                        w1_lhsT(k, j),
                        xts[:, k * ST:(k + 1) * ST],
                        start=(k == 0),
                        stop=(k == KC - 1),
                    )
                tts[j] = ttp.tile([P, ST], f32r, tag="tt", name=f"tt{b}_{st}_{j}")
                nc.scalar.activation(
                    tts[j][:], pj[:], AF.Tanh, bias=bias_sb[j][:, b:b + 1]
                )
                if j >= 1:
                    matvec(j - 1)
            matvec(UC - 1)
            nc.tensor.matmul(spp[:], one_sb[:], acc[:], start=True, stop=True)
            return spp

        def exp_scatter(b, st, spp, lo=0, hi=ST):
            """exp (+partial sum) of score columns [lo, hi); scatter so that
            pcol[p, t] = piece[p*tp + t] (partition-major within the piece)."""
            pr, zp, _ = batch_state(b)
            n = hi - lo
            tp = n // P
            zslot = st if lo == 0 else NST + st % NST
            nc.scalar.activation(
                pr[:, st * ST + lo:st * ST + hi],
                spp[:] if (lo == 0 and hi == ST) else spp[:, 0:n],
                AF.Exp,
                accum_out=zp[:, zslot:zslot + 1],
            )
            pbt = dramp.tile([1, n], f32, tag="pb", name=f"pb{b}_{st}_{lo}")
            nc.gpsimd.dma_start(out=pbt[:], in_=pr[:, st * ST + lo:st * ST + hi])
            pcol = smallp.tile([P, tp], f32r, tag="pcol", name=f"pc{b}_{st}_{lo}")
            nc.gpsimd.dma_start(
                out=pcol[:],
                in_=pbt[:].rearrange("a (p t) -> p (a t)", p=P).bitcast(f32r),
            )
            return pcol

        def ctx_mms(b, st, pcol, tp=TPT, piece_lo=None, first=False, last=False):
            """context matmuls (unnormalized weights) for one s-tile or piece."""
            _, _, cps = batch_state(b)
            if piece_lo is None:
                # rows follow the whole-tile mapping s = st*ST + p*TPT + t
                nv = nvp.tile([P, TPT * D], f32r, tag="nv", name=f"nv{b}_{st}")
                nc.sync.dma_start(
                    out=nv[:],
                    in_=val[b]
                    .rearrange("(g p t) d -> g p (t d)", p=P, t=TPT)[st]
                    .bitcast(f32r),
                )
            else:
                # piece mapping: rows s = st*ST + piece_lo + p*tp + t
                nv = nvp.tile([P, tp * D], f32r, tag="nv", name=f"nv{b}_{st}_{piece_lo}")
                nc.sync.dma_start(
                    out=nv[:],
                    in_=val[b, st * ST + piece_lo:st * ST + piece_lo + tp * P, :]
                    .rearrange("(p t) d -> p (t d)", p=P)
                    .bitcast(f32r),
                )
            for tloc in range(tp):
                for dn in range(2):
                    nc.tensor.matmul(
                        cps[dn][:],
                        pcol[:, tloc:tloc + 1],
                        nv[:, tloc * D + dn * ST:tloc * D + (dn + 1) * ST],
                        start=(first and tloc == 0),
                        stop=(last and tloc == tp - 1),
                    )

        def finalize(b):
            """1/Z normalization of both outputs; DMA out."""
            pr, zp, cps = batch_state(b)
            z = smallp.tile([1, 1], f32, tag="z", name=f"z{b}")
            nc.vector.reduce_sum(out=z[:], in_=zp[:], axis=AX.X)
            rz = smallp.tile([1, 1], f32, tag="rz", name=f"rz{b}")
            nc.vector.reciprocal(rz[:], z[:])
            at = rowp.tile([1, S], f32, tag="prow", name=f"at{b}")
            nc.vector.tensor_scalar_mul(out=at[:], in0=pr[:], scalar1=rz[:, 0:1])
            nc.gpsimd.dma_start(out=oattn[b:b + 1, :], in_=at[:])
            crow = smallp.tile([1, D], f32, tag="crow", name=f"cr{b}", bufs=2)
            for dn in range(2):
                nc.vector.tensor_scalar_mul(
                    out=crow[:, dn * ST:(dn + 1) * ST], in0=cps[dn][:], scalar1=rz[:, 0:1]
                )
            nc.gpsimd.dma_start(out=octx[b:b + 1, :], in_=crow[:])
            del state[b]

        # s-tile software pipeline: ctx matmuls of tile i run after the score
        # matmuls of tile i+1, so the PE never waits on exp/scatter.
        tasks = [(b, st) for b in range(NB) for st in range(NST)]
        pend = None
        for idx, (b, st) in enumerate(tasks):
            last = idx == len(tasks) - 1
            if b == 0 and st == 0:
                spp = score_stile(
                    b, st, xts=first_xts,
                    pre_j=lambda j: (proj_q(j) if j >= UC // 2 else None),
                )
            else:
                spp = score_stile(b, st)
            if last:
                # emit the final exp/scatter before pend's nv DMAs so the
                # tail-critical scatter isn't queued behind bulk traffic
                mine = exp_scatter(b, st, spp)
            if pend is not None:
                pb_, pst_, pcol_ = pend
                ctx_mms(pb_, pst_, pcol_, first=(pst_ == 0),
                        last=(pst_ == NST - 1))
                if pst_ == NST - 1:
                    finalize(pb_)
            pend = (b, st, exp_scatter(b, st, spp)) if not last else (b, st, mine)
        pb_, pst_, pcol_ = pend
        # keep the PE busy (and the HAM clock warm) while the final
        # exp/scatter chain completes
        tdps = pp.tile([P, ST], f32, tag="proj", name="tdps")
        for i in range(12):
            nc.tensor.matmul(tdps[:], zt[:, 0:P], zt[:], start=True, stop=True)
        ctx_mms(pb_, pst_, pcol_, first=(pst_ == 0), last=(pst_ == NST - 1))
        finalize(pb_)

    nc.compile()
    return nc


def kernel(query, values, W1, b1, W2, b2, V, bV, _trace=False, _trace_kwargs=None):
    from concourse.bass_utils import run_bass_kernel_spmd

    query = np.asarray(query, dtype=np.float32)
    values = np.asarray(values, dtype=np.float32)
    W1 = np.asarray(W1, dtype=np.float32)
    b1 = np.asarray(b1, dtype=np.float32)
    W2 = np.asarray(W2, dtype=np.float32)
    b2 = np.asarray(b2, dtype=np.float32)
    V = np.asarray(V, dtype=np.float32)

    assert query.shape == (B, D) and values.shape == (B, S, D)

    if "nc" not in _CACHE:
        _CACHE["nc"] = _build()
    nc = _CACHE["nc"]

    valuesT = np.ascontiguousarray(values.transpose(0, 2, 1))  # [B, D, S]
    qTf = np.ascontiguousarray(query.T)                        # [D, B]
    bcf = np.ascontiguousarray((b1 + b2).reshape(U, 1))
    Vf = np.ascontiguousarray(V.reshape(U, 1))

    in_maps = []
    for c in range(NC):
        lo, hi = c * NB, (c + 1) * NB
        in_maps.append({
            "xt": valuesT[lo:hi],
            "val": values[lo:hi],
            "qT": np.ascontiguousarray(qTf[:, lo:hi]),
            "w1": W1,
            "w2": W2,
            "bc": bcf,
            "vv": Vf,
            "one": np.ones((P, 1), np.float32),
        })

    try:
        res = run_bass_kernel_spmd(
            nc, in_maps, list(range(NC)), trace=_trace, **(_trace_kwargs or {})
        )
    except Exception:
        # transient device/axon flake: retry once
        res = run_bass_kernel_spmd(
            nc, in_maps, list(range(NC)), trace=_trace, **(_trace_kwargs or {})
        )
    _CACHE["last_result"] = res

    context = np.concatenate([res.results[c]["octx"] for c in range(NC)], axis=0)
    attn = np.concatenate([res.results[c]["oattn"] for c in range(NC)], axis=0)
    return context, attn.reshape(B, S, 1)


# revision 28
# speedup vs baseline: 1.1092x; 1.0086x over previous
"""Bahdanau attention on 8 Trainium2 NeuronCores (Bass/Tile).

reference:
    proj_v = values @ W1 + b1             # [B, S, U]
    proj_q = (query @ W2 + b2)[:, None]   # [B, 1, U]
    score  = tanh(proj_v + proj_q) @ V + bV
    attn   = softmax(score, axis=1)       # [B, S, 1]
    ctx    = sum(attn * values, axis=1)   # [B, D]

Sharding: data-parallel over batch B=32 across 8 cores (4 batches/core);
each core holds full W1/W2/V.

Device-side strategy: the big matmul (values @ W1) contracts over d,
which must live on SBUF partitions for the PE.  values arrives from HBM
in natural [s, d] layout, so the host ships a second, pre-transposed
copy valuesT [d, s] (pure layout prep, same bytes) and the kernel never
transposes on-chip:
  - scores:  psum[u,s] += W1[dchunk,uchunk].T @ valuesT[dchunk, stile]
             (float32r = fp32 bits at full PE rate), tanh+bias fused in
             one ScalarE activation (bias = (b1+b2+query@W2)[u] is
             per-partition in this orientation).  The V projection runs
             on the Vector engine (per-partition multiply-accumulate)
             with a single ones-column matmul per s-tile doing the
             cross-partition reduce, keeping 7 of 8 matvec matmuls off
             the saturated PE.  bV is dropped: softmax is
             shift-invariant.
  - softmax: flash-style without max subtraction (scores for this
             model/data are O(+-3); exp cannot overflow fp32): exp+sum
             fused in one activation(accum_out=...) per s-tile, ctx
             accumulated with UNNORMALIZED weights, one 1/Z scale at
             batch end for both outputs.
  - context: ctx[1,d] += p_col[schunk].T @ values[schunk, d] with
             natural-layout tiles; the exp row is bounced through DRAM
             to scatter it across partitions (partition-major mapping so
             the scatter reads 16B-contiguous per partition).  Context
             matmuls for s-tile i are emitted after the score matmuls of
             s-tile i+1 so the PE never waits on the exp/scatter chain.

Startup: W2 is loaded as per-uchunk column tiles and the tiny proj_q
matmul groups are interleaved into the first s-tile's j-loop; dummy
matmuls on a zeroed tile warm the PE HAM clock gate while W1/xt stream
in.
"""

import numpy as np

B, S, D, U = 32, 2048, 1024, 1024
NC = 8
NB = B // NC          # batches per core
P = 128
KC = D // P           # contraction chunks
UC = U // P           # units chunks
NST = 4               # score s-tiles per batch
ST = S // NST         # 512
TPT = ST // P         # context s-blocks per s-tile (4)
WARMUP_MMS = 16

_CACHE = {}


def _build():
    from contextlib import ExitStack

    import concourse.bacc as bacc
    import concourse.tile as tile
    from concourse import mybir

    f32 = mybir.dt.float32
    f32r = mybir.dt.float32r
    AF = mybir.ActivationFunctionType
    AX = mybir.AxisListType

    nc = bacc.Bacc("TRN2", target_bir_lowering=False, debug=False, num_devices=NC)

    xt = nc.declare_dram_parameter("xt", [NB, D, S], f32, isOutput=False)
    val = nc.declare_dram_parameter("val", [NB, S, D], f32, isOutput=False)
    qT = nc.declare_dram_parameter("qT", [D, NB], f32, isOutput=False)
    w1 = nc.declare_dram_parameter("w1", [D, U], f32, isOutput=False)
    w2 = nc.declare_dram_parameter("w2", [D, U], f32, isOutput=False)
    bc = nc.declare_dram_parameter("bc", [U, 1], f32, isOutput=False)
    vv = nc.declare_dram_parameter("vv", [U, 1], f32, isOutput=False)
    one = nc.declare_dram_parameter("one", [P, 1], f32, isOutput=False)
    octx = nc.declare_dram_parameter("octx", [NB, D], f32, isOutput=True)
    oattn = nc.declare_dram_parameter("oattn", [NB, S], f32, isOutput=True)

    with tile.TileContext(nc) as tc, ExitStack() as ctx:
        consts = ctx.enter_context(tc.tile_pool(name="consts", bufs=1))
        xtp = ctx.enter_context(tc.tile_pool(name="xtp", bufs=2))
        ttp = ctx.enter_context(tc.tile_pool(name="ttp", bufs=4))
        accp = ctx.enter_context(tc.tile_pool(name="accp", bufs=2))
        nvp = ctx.enter_context(tc.tile_pool(name="nvp", bufs=2))
        rowp = ctx.enter_context(tc.tile_pool(name="rowp", bufs=3))
        smallp = ctx.enter_context(tc.tile_pool(name="smallp", bufs=3))
        pp = ctx.enter_context(tc.tile_pool(name="pp", bufs=3, space="PSUM"))
        sppp = ctx.enter_context(tc.tile_pool(name="sppp", bufs=2, space="PSUM"))
        ctxp = ctx.enter_context(tc.tile_pool(name="ctxp", bufs=3, space="PSUM"))
        dramp = ctx.enter_context(tc.tile_pool(name="dramp", bufs=3, space="DRAM"))

        # ---- prologue DMAs, batched wide (one dma_start spreads across all
        # 16 SDMA engines; >=1MiB hits ~78%+ of peak) and ordered by when the
        # PE needs the bytes ----
        qt_all = consts.tile([P, KC * NB], f32r, tag="qt", name="qt_all")
        nc.sync.dma_start(
            out=qt_all[:].rearrange("p (k b) -> p k b", k=KC),
            in_=qT[:].rearrange("(k p) b -> p k b", p=P).bitcast(f32r),
        )
        bc_all = consts.tile([P, KC], f32, tag="bc", name="bc_all")
        nc.sync.dma_start(
            out=bc_all[:].rearrange("p (k a) -> p k a", k=KC),
            in_=bc[:].rearrange("(k p) a -> p k a", p=P),
        )
        v_all = consts.tile([P, KC], f32r, tag="v", name="v_all")
        nc.sync.dma_start(
            out=v_all[:].rearrange("p (k a) -> p k a", k=KC),
            in_=vv[:].rearrange("(k p) a -> p k a", p=P).bitcast(f32r),
        )
        one_sb = consts.tile([P, 1], f32r, tag="one", name="one_sb")
        nc.sync.dma_start(out=one_sb[:], in_=one[:].bitcast(f32r))
        qt_sb = [qt_all[:, k * NB:(k + 1) * NB] for k in range(KC)]
        bc_sb = [bc_all[:, k:k + 1] for k in range(KC)]
        v_sb = [v_all[:, k:k + 1] for k in range(KC)]

        UH = U // 2

        def load_half(dst_handle, h, tag, name):
            t = consts.tile([P, KC * UH], f32r, tag=tag, name=name)
            nc.sync.dma_start(
                out=t[:].rearrange("p (k u) -> p k u", k=KC),
                in_=dst_handle[:, h * UH:(h + 1) * UH]
                .rearrange("(k p) u -> p k u", p=P)
                .bitcast(f32r),
            )
            return t

        UQ = U // 4

        def load_quarter(src_handle, q, tag, name):
            t = consts.tile([P, KC * UQ], f32r, tag=tag, name=name)
            nc.sync.dma_start(
                out=t[:].rearrange("p (k u) -> p k u", k=KC),
                in_=src_handle[:, q * UQ:(q + 1) * UQ]
                .rearrange("(k p) u -> p k u", p=P)
                .bitcast(f32r),
            )
            return t

        # DMA order = the order the PE consumes the bytes:
        #   w2h0 (proj_q j0-3) -> w1q0/q1+xt00 (scores j0-3) -> w1q2/q3
        #   (j4-7, staggered) -> w2h1 (proj_q j4-7) -> steady state
        w2h = [None, None]
        w1q = [None] * 4
        w2h[0] = load_half(w2, 0, "w2_0", "w2h0")
        w1q[0] = load_quarter(w1, 0, "w1_0", "w1q0")
        w1q[1] = load_quarter(w1, 1, "w1_1", "w1q1")
        first_xts = xtp.tile([P, KC * ST], f32r, tag="xt", name="xt0_0")
        nc.sync.dma_start(
            out=first_xts[:].rearrange("p (k s) -> p k s", k=KC),
            in_=xt[0, :, 0:ST]
            .rearrange("(k p) s -> p k s", p=P)
            .bitcast(f32r),
        )
        w1q[2] = load_quarter(w1, 2, "w1_2", "w1q2")
        w1q[3] = load_quarter(w1, 3, "w1_3", "w1q3")
        w2h[1] = load_half(w2, 1, "w2_1", "w2h1")

        def w1_lhsT(k, j):
            q, jj = divmod(j, UC // 4)
            return w1q[q][:, k * UQ + jj * P:k * UQ + (jj + 1) * P]

        bias_sb = [None] * UC

        def proj_q(j):
            qp = pp.tile([P, NB], f32, tag="proj", name=f"qp{j}")
            h, jj = divmod(j, UC // 2)
            for k in range(KC):
                nc.tensor.matmul(
                    qp[:],
                    w2h[h][:, k * UH + jj * P:k * UH + (jj + 1) * P],
                    qt_sb[k],
                    start=(k == 0), stop=(k == KC - 1),
                )
            bt = consts.tile([P, NB], f32, tag=f"bias_{j}", name=f"bias{j}")
            nc.vector.tensor_scalar_add(out=bt[:], in0=qp[:], scalar1=bc_sb[j])
            bias_sb[j] = bt

        # Warm the PE HAM clock gate while W2/W1/xt stream from HBM.  Plain
        # fp32 matmuls run 4 cycles/row, so a few fill the warmup window.
        zt = consts.tile([P, ST], f32, tag="zt", name="zt")
        nc.vector.memset(zt[:], 0.0)
        dps = pp.tile([P, ST], f32, tag="proj", name="dps")
        for i in range(WARMUP_MMS):
            nc.tensor.matmul(dps[:], zt[:, 0:P], zt[:], start=True, stop=True)

        for j in range(UC // 2):
            proj_q(j)

        # ---- per-(batch, s-tile) stages ----
        state = {}  # per-batch: pr row, zp, cps accumulators

        def batch_state(b):
            if b not in state:
                pr = rowp.tile([1, S], f32, tag="prow", name=f"pr{b}")
                zp = smallp.tile([1, 2 * NST], f32, tag="zp", name=f"zp{b}")
                nc.vector.memset(zp[:], 0.0)
                cps = [
                    ctxp.tile([1, ST], f32, tag="ctx", name=f"cp{b}_{dn}")
                    for dn in range(2)
                ]
                state[b] = (pr, zp, cps)
            return state[b]

        def score_stile(b, st, xts=None, pre_j=None):
            """64 proj matmuls + 8 tanh for one s-tile; the V matvec runs on
            DVE (per-partition multiply) + GpSimd (cross-partition reduce) to
            keep those 8 N=512 matmuls off the PE."""
            if xts is None:
                xts = xtp.tile([P, KC * ST], f32r, tag="xt", name=f"xt{b}_{st}")
                nc.sync.dma_start(
                    out=xts[:].rearrange("p (k s) -> p k s", k=KC),
                    in_=xt[b, :, st * ST:(st + 1) * ST]
                    .rearrange("(k p) s -> p k s", p=P)
                    .bitcast(f32r),
                )
            acc = accp.tile([P, ST], f32r, tag="acc", name=f"acc{b}_{st}")
            tmp = accp.tile([P, ST], f32, tag="tmp", name=f"tmp{b}_{st}")
            spp = sppp.tile([1, ST], f32, tag="spp", name=f"spp{b}_{st}")
            tts = [None] * UC

            def matvec(j):
                vj = v_all[:, j:j + 1].bitcast(f32)
                if j == 0:
                    nc.vector.tensor_scalar_mul(
                        out=acc[:], in0=tts[j][:].bitcast(f32), scalar1=vj
                    )
                else:
                    nc.vector.tensor_scalar_mul(
                        out=tmp[:], in0=tts[j][:].bitcast(f32), scalar1=vj
                    )
                    nc.vector.tensor_tensor(
                        out=acc[:], in0=acc[:].bitcast(f32), in1=tmp[:],
                        op=mybir.AluOpType.add,
                    )

            for j in range(UC):
                if pre_j is not None:
                    pre_j(j)
                pj = pp.tile([P, ST], f32, tag="proj", name=f"pj{b}_{st}_{j}")
                for k in range(KC):
                    nc.tensor.matmul(
                        pj[:],
                        w1_lhsT(k, j),
                        xts[:, k * ST:(k + 1) * ST],
                        start=(k == 0),
                        stop=(k == KC - 1),
                    )
                tts[j] = ttp.tile([P, ST], f32r, tag="tt", name=f"tt{b}_{st}_{j}")
                nc.scalar.activation(
                    tts[j][:], pj[:], AF.Tanh, bias=bias_sb[j][:, b:b + 1]
                )
                if j >= 1:
                    matvec(j - 1)
            matvec(UC - 1)
            nc.tensor.matmul(spp[:], one_sb[:], acc[:], start=True, stop=True)
            return spp

        def exp_scatter(b, st, spp, lo=0, hi=ST):
            """exp (+partial sum) of score columns [lo, hi); scatter so that
            pcol[p, t] = piece[p*tp + t] (partition-major within the piece)."""
            pr, zp, _ = batch_state(b)
            n = hi - lo
            tp = n // P
            zslot = st if lo == 0 else NST + st % NST
            nc.scalar.activation(
                pr[:, st * ST + lo:st * ST + hi],
                spp[:] if (lo == 0 and hi == ST) else spp[:, 0:n],
                AF.Exp,
                accum_out=zp[:, zslot:zslot + 1],
            )
            pbt = dramp.tile([1, n], f32, tag="pb", name=f"pb{b}_{st}_{lo}")
            nc.gpsimd.dma_start(out=pbt[:], in_=pr[:, st * ST + lo:st * ST + hi])
            pcol = smallp.tile([P, tp], f32r, tag="pcol", name=f"pc{b}_{st}_{lo}")
            nc.gpsimd.dma_start(
                out=pcol[:],
                in_=pbt[:].rearrange("a (p t) -> p (a t)", p=P).bitcast(f32r),
            )
            return pcol

        def ctx_mms(b, st, pcol, tp=TPT, piece_lo=None, first=False, last=False):
            """context matmuls (unnormalized weights) for one s-tile or piece."""
            _, _, cps = batch_state(b)
            if piece_lo is None:
                # rows follow the whole-tile mapping s = st*ST + p*TPT + t
                nv = nvp.tile([P, TPT * D], f32r, tag="nv", name=f"nv{b}_{st}")
                nc.sync.dma_start(
                    out=nv[:],
                    in_=val[b]
                    .rearrange("(g p t) d -> g p (t d)", p=P, t=TPT)[st]
                    .bitcast(f32r),
                )
            else:
                # piece mapping: rows s = st*ST + piece_lo + p*tp + t
                nv = nvp.tile([P, tp * D], f32r, tag="nv", name=f"nv{b}_{st}_{piece_lo}")
                nc.sync.dma_start(
                    out=nv[:],
                    in_=val[b, st * ST + piece_lo:st * ST + piece_lo + tp * P, :]
                    .rearrange("(p t) d -> p (t d)", p=P)
                    .bitcast(f32r),
                )
            for tloc in range(tp):
                for dn in range(2):
                    nc.tensor.matmul(
                        cps[dn][:],
                        pcol[:, tloc:tloc + 1],
                        nv[:, tloc * D + dn * ST:tloc * D + (dn + 1) * ST],
                        start=(first and tloc == 0),
                        stop=(last and tloc == tp - 1),
                    )

        def finalize(b):
            """1/Z normalization of both outputs; DMA out."""
            pr, zp, cps = batch_state(b)
            z = smallp.tile([1, 1], f32, tag="z", name=f"z{b}")
            nc.vector.reduce_sum(out=z[:], in_=zp[:], axis=AX.X)
            rz = smallp.tile([1, 1], f32, tag="rz", name=f"rz{b}")
            nc.vector.reciprocal(rz[:], z[:])
            at = rowp.tile([1, S], f32, tag="prow", name=f"at{b}")
            nc.vector.tensor_scalar_mul(out=at[:], in0=pr[:], scalar1=rz[:, 0:1])
            nc.gpsimd.dma_start(out=oattn[b:b + 1, :], in_=at[:])
            crow = smallp.tile([1, D], f32, tag="crow", name=f"cr{b}", bufs=2)
            for dn in range(2):
                nc.vector.tensor_scalar_mul(
                    out=crow[:, dn * ST:(dn + 1) * ST], in0=cps[dn][:], scalar1=rz[:, 0:1]
                )
            nc.gpsimd.dma_start(out=octx[b:b + 1, :], in_=crow[:])
            del state[b]

        # s-tile software pipeline: ctx matmuls of tile i run after the score
        # matmuls of tile i+1, so the PE never waits on exp/scatter.
        tasks = [(b, st) for b in range(NB) for st in range(NST)]
        pend = None
        for idx, (b, st) in enumerate(tasks):
            last = idx == len(tasks) - 1
            if b == 0 and st == 0:
                spp = score_stile(
                    b, st, xts=first_xts,
                    pre_j=lambda j: (proj_q(j) if j >= UC // 2 else None),
                )
            else:
                spp = score_stile(b, st)
            if last:
                # emit the final exp/scatter before pend's nv DMAs so the
                # tail-critical scatter isn't queued behind bulk traffic
                mine = exp_scatter(b, st, spp)
            if pend is not None:
                pb_, pst_, pcol_ = pend
                ctx_mms(pb_, pst_, pcol_, first=(pst_ == 0),
                        last=(pst_ == NST - 1))
                if pst_ == NST - 1:
                    finalize(pb_)
            pend = (b, st, exp_scatter(b, st, spp)) if not last else (b, st, mine)
        pb_, pst_, pcol_ = pend
        # keep the PE busy (and the HAM clock warm) while the final
        # exp/scatter chain completes
        tdps = pp.tile([P, ST], f32, tag="proj", name="tdps")
        for i in range(12):
            nc.tensor.matmul(tdps[:], zt[:, 0:P], zt[:], start=True, stop=True)
        ctx_mms(pb_, pst_, pcol_, first=(pst_ == 0), last=(pst_ == NST - 1))
        finalize(pb_)

    nc.compile()
    return nc


def kernel(query, values, W1, b1, W2, b2, V, bV, _trace=False, _trace_kwargs=None):
    from concourse.bass_utils import run_bass_kernel_spmd

    query = np.asarray(query, dtype=np.float32)
    values = np.asarray(values, dtype=np.float32)
    W1 = np.asarray(W1, dtype=np.float32)
    b1 = np.asarray(b1, dtype=np.float32)
    W2 = np.asarray(W2, dtype=np.float32)
    b2 = np.asarray(b2, dtype=np.float32)
    V = np.asarray(V, dtype=np.float32)

    assert query.shape == (B, D) and values.shape == (B, S, D)

    if "nc" not in _CACHE:
        _CACHE["nc"] = _build()
    nc = _CACHE["nc"]

    valuesT = np.ascontiguousarray(values.transpose(0, 2, 1))  # [B, D, S]
    qTf = np.ascontiguousarray(query.T)                        # [D, B]
    bcf = np.ascontiguousarray((b1 + b2).reshape(U, 1))
    Vf = np.ascontiguousarray(V.reshape(U, 1))

    in_maps = []
    for c in range(NC):
        lo, hi = c * NB, (c + 1) * NB
        in_maps.append({
            "xt": valuesT[lo:hi],
            "val": values[lo:hi],
            "qT": np.ascontiguousarray(qTf[:, lo:hi]),
            "w1": W1,
            "w2": W2,
            "bc": bcf,
            "vv": Vf,
            "one": np.ones((P, 1), np.float32),
        })

    try:
        res = run_bass_kernel_spmd(
            nc, in_maps, list(range(NC)), trace=_trace, **(_trace_kwargs or {})
        )
    except Exception:
        # transient device/axon flake: retry once
        res = run_bass_kernel_spmd(
            nc, in_maps, list(range(NC)), trace=_trace, **(_trace_kwargs or {})
        )
    _CACHE["last_result"] = res

    context = np.concatenate([res.results[c]["octx"] for c in range(NC)], axis=0)
    attn = np.concatenate([res.results[c]["oattn"] for c in range(NC)], axis=0)
    return context, attn.reshape(B, S, 1)


# revision 30
# speedup vs baseline: 1.1121x; 1.0026x over previous
"""Bahdanau attention on 8 Trainium2 NeuronCores (Bass/Tile).

reference:
    proj_v = values @ W1 + b1             # [B, S, U]
    proj_q = (query @ W2 + b2)[:, None]   # [B, 1, U]
    score  = tanh(proj_v + proj_q) @ V + bV
    attn   = softmax(score, axis=1)       # [B, S, 1]
    ctx    = sum(attn * values, axis=1)   # [B, D]

Sharding: data-parallel over batch B=32 across 8 cores (4 batches/core);
each core holds full W1/W2/V.

Device-side strategy: the big matmul (values @ W1) contracts over d,
which must live on SBUF partitions for the PE.  values arrives from HBM
in natural [s, d] layout, so the host ships a second, pre-transposed
copy valuesT [d, s] (pure layout prep, same bytes) and the kernel never
transposes on-chip:
  - scores:  psum[u,s] += W1[dchunk,uchunk].T @ valuesT[dchunk, stile]
             (float32r = fp32 bits at full PE rate), tanh+bias fused in
             one ScalarE activation (bias = (b1+b2+query@W2)[u] is
             per-partition in this orientation).  The V projection runs
             on the Vector engine (per-partition multiply-accumulate)
             with a single ones-column matmul per s-tile doing the
             cross-partition reduce, keeping 7 of 8 matvec matmuls off
             the saturated PE.  bV is dropped: softmax is
             shift-invariant.
  - softmax: flash-style without max subtraction (scores for this
             model/data are O(+-3); exp cannot overflow fp32): exp+sum
             fused in one activation(accum_out=...) per s-tile, ctx
             accumulated with UNNORMALIZED weights, one 1/Z scale at
             batch end for both outputs.
  - context: ctx[1,d] += p_col[schunk].T @ values[schunk, d] with
             natural-layout tiles; the exp row is bounced through DRAM
             to scatter it across partitions (partition-major mapping so
             the scatter reads 16B-contiguous per partition).  Context
             matmuls for s-tile i are emitted after the score matmuls of
             s-tile i+1 so the PE never waits on the exp/scatter chain.

Startup: W2 is loaded as per-uchunk column tiles and the tiny proj_q
matmul groups are interleaved into the first s-tile's j-loop; dummy
matmuls on a zeroed tile warm the PE HAM clock gate while W1/xt stream
in.
"""

import numpy as np

B, S, D, U = 32, 2048, 1024, 1024
NC = 8
NB = B // NC          # batches per core
P = 128
KC = D // P           # contraction chunks
UC = U // P           # units chunks
NST = 4               # score s-tiles per batch
ST = S // NST         # 512
TPT = ST // P         # context s-blocks per s-tile (4)
WARMUP_MMS = 16

_CACHE = {}


def _build():
    from contextlib import ExitStack

    import concourse.bacc as bacc
    import concourse.tile as tile
    from concourse import mybir

    f32 = mybir.dt.float32
    f32r = mybir.dt.float32r
    AF = mybir.ActivationFunctionType
    AX = mybir.AxisListType

    nc = bacc.Bacc("TRN2", target_bir_lowering=False, debug=False, num_devices=NC)

    xt = nc.declare_dram_parameter("xt", [NB, D, S], f32, isOutput=False)
    val = nc.declare_dram_parameter("val", [NB, S, D], f32, isOutput=False)
    qT = nc.declare_dram_parameter("qT", [D, NB], f32, isOutput=False)
    w1 = nc.declare_dram_parameter("w1", [D, U], f32, isOutput=False)
    w2 = nc.declare_dram_parameter("w2", [D, U], f32, isOutput=False)
    bc = nc.declare_dram_parameter("bc", [U, 1], f32, isOutput=False)
    vv = nc.declare_dram_parameter("vv", [U, 1], f32, isOutput=False)
    one = nc.declare_dram_parameter("one", [P, 1], f32, isOutput=False)
    octx = nc.declare_dram_parameter("octx", [NB, D], f32, isOutput=True)
    oattn = nc.declare_dram_parameter("oattn", [NB, S], f32, isOutput=True)

    with tile.TileContext(nc) as tc, ExitStack() as ctx:
        consts = ctx.enter_context(tc.tile_pool(name="consts", bufs=1))
        xtp = ctx.enter_context(tc.tile_pool(name="xtp", bufs=2))
        ttp = ctx.enter_context(tc.tile_pool(name="ttp", bufs=4))
        accp = ctx.enter_context(tc.tile_pool(name="accp", bufs=2))
        nvp = ctx.enter_context(tc.tile_pool(name="nvp", bufs=2))
        rowp = ctx.enter_context(tc.tile_pool(name="rowp", bufs=3))
        smallp = ctx.enter_context(tc.tile_pool(name="smallp", bufs=3))
        pp = ctx.enter_context(tc.tile_pool(name="pp", bufs=3, space="PSUM"))
        sppp = ctx.enter_context(tc.tile_pool(name="sppp", bufs=2, space="PSUM"))
        ctxp = ctx.enter_context(tc.tile_pool(name="ctxp", bufs=3, space="PSUM"))
        dramp = ctx.enter_context(tc.tile_pool(name="dramp", bufs=3, space="DRAM"))

        # ---- prologue DMAs, batched wide (one dma_start spreads across all
        # 16 SDMA engines; >=1MiB hits ~78%+ of peak) and ordered by when the
        # PE needs the bytes ----
        qt_all = consts.tile([P, KC * NB], f32r, tag="qt", name="qt_all")
        nc.sync.dma_start(
            out=qt_all[:].rearrange("p (k b) -> p k b", k=KC),
            in_=qT[:].rearrange("(k p) b -> p k b", p=P).bitcast(f32r),
        )
        bc_all = consts.tile([P, KC], f32, tag="bc", name="bc_all")
        nc.sync.dma_start(
            out=bc_all[:].rearrange("p (k a) -> p k a", k=KC),
            in_=bc[:].rearrange("(k p) a -> p k a", p=P),
        )
        v_all = consts.tile([P, KC], f32r, tag="v", name="v_all")
        nc.sync.dma_start(
            out=v_all[:].rearrange("p (k a) -> p k a", k=KC),
            in_=vv[:].rearrange("(k p) a -> p k a", p=P).bitcast(f32r),
        )
        one_sb = consts.tile([P, 1], f32r, tag="one", name="one_sb")
        nc.sync.dma_start(out=one_sb[:], in_=one[:].bitcast(f32r))
        qt_sb = [qt_all[:, k * NB:(k + 1) * NB] for k in range(KC)]
        bc_sb = [bc_all[:, k:k + 1] for k in range(KC)]
        v_sb = [v_all[:, k:k + 1] for k in range(KC)]

        UH = U // 2

        def load_half(dst_handle, h, tag, name):
            t = consts.tile([P, KC * UH], f32r, tag=tag, name=name)
            nc.sync.dma_start(
                out=t[:].rearrange("p (k u) -> p k u", k=KC),
                in_=dst_handle[:, h * UH:(h + 1) * UH]
                .rearrange("(k p) u -> p k u", p=P)
                .bitcast(f32r),
            )
            return t

        UQ = U // 4

        def load_quarter(src_handle, q, tag, name):
            t = consts.tile([P, KC * UQ], f32r, tag=tag, name=name)
            nc.sync.dma_start(
                out=t[:].rearrange("p (k u) -> p k u", k=KC),
                in_=src_handle[:, q * UQ:(q + 1) * UQ]
                .rearrange("(k p) u -> p k u", p=P)
                .bitcast(f32r),
            )
            return t

        # DMA order = the order the PE consumes the bytes:
        #   w2h0 (proj_q j0-3) -> w1q0/q1+xt00 (scores j0-3) -> w1q2/q3
        #   (j4-7, staggered) -> w2h1 (proj_q j4-7) -> steady state
        w2h = [None, None]
        w1q = [None] * 4
        w2h[0] = load_half(w2, 0, "w2_0", "w2h0")
        w1q[0] = load_quarter(w1, 0, "w1_0", "w1q0")
        w1q[1] = load_quarter(w1, 1, "w1_1", "w1q1")
        first_xts = xtp.tile([P, KC * ST], f32r, tag="xt", name="xt0_0")
        nc.sync.dma_start(
            out=first_xts[:].rearrange("p (k s) -> p k s", k=KC),
            in_=xt[0, :, 0:ST]
            .rearrange("(k p) s -> p k s", p=P)
            .bitcast(f32r),
        )
        w1q[2] = load_quarter(w1, 2, "w1_2", "w1q2")
        w1q[3] = load_quarter(w1, 3, "w1_3", "w1q3")
        w2h[1] = load_half(w2, 1, "w2_1", "w2h1")

        def w1_lhsT(k, j):
            q, jj = divmod(j, UC // 4)
            return w1q[q][:, k * UQ + jj * P:k * UQ + (jj + 1) * P]

        bias_sb = [None] * UC

        def proj_q(j):
            qp = pp.tile([P, NB], f32, tag="proj", name=f"qp{j}")
            h, jj = divmod(j, UC // 2)
            for k in range(KC):
                nc.tensor.matmul(
                    qp[:],
                    w2h[h][:, k * UH + jj * P:k * UH + (jj + 1) * P],
                    qt_sb[k],
                    start=(k == 0), stop=(k == KC - 1),
                )
            bt = consts.tile([P, NB], f32, tag=f"bias_{j}", name=f"bias{j}")
            nc.vector.tensor_scalar_add(out=bt[:], in0=qp[:], scalar1=bc_sb[j])
            bias_sb[j] = bt

        # Warm the PE HAM clock gate while W2/W1/xt stream from HBM.  Plain
        # fp32 matmuls run 4 cycles/row, so a few fill the warmup window.
        zt = consts.tile([P, ST], f32, tag="zt", name="zt")
        nc.vector.memset(zt[:], 0.0)
        dps = pp.tile([P, ST], f32, tag="proj", name="dps")
        for i in range(WARMUP_MMS):
            nc.tensor.matmul(dps[:], zt[:, 0:P], zt[:], start=True, stop=True)

        for j in range(UC // 2):
            proj_q(j)

        # ---- per-(batch, s-tile) stages ----
        state = {}  # per-batch: pr row, zp, cps accumulators

        def batch_state(b):
            if b not in state:
                pr = rowp.tile([1, S], f32, tag="prow", name=f"pr{b}")
                zp = smallp.tile([1, 2 * NST], f32, tag="zp", name=f"zp{b}")
                nc.vector.memset(zp[:], 0.0)
                cps = [
                    ctxp.tile([1, ST], f32, tag="ctx", name=f"cp{b}_{dn}")
                    for dn in range(2)
                ]
                state[b] = (pr, zp, cps)
            return state[b]

        def score_stile(b, st, xts=None, pre_j=None):
            """64 proj matmuls + 8 tanh for one s-tile; the V matvec runs on
            DVE (per-partition multiply) + GpSimd (cross-partition reduce) to
            keep those 8 N=512 matmuls off the PE."""
            if xts is None:
                xts = xtp.tile([P, KC * ST], f32r, tag="xt", name=f"xt{b}_{st}")
                nc.sync.dma_start(
                    out=xts[:].rearrange("p (k s) -> p k s", k=KC),
                    in_=xt[b, :, st * ST:(st + 1) * ST]
                    .rearrange("(k p) s -> p k s", p=P)
                    .bitcast(f32r),
                )
            acc = accp.tile([P, ST], f32r, tag="acc", name=f"acc{b}_{st}")
            tmp = accp.tile([P, ST], f32, tag="tmp", name=f"tmp{b}_{st}")
            spp = sppp.tile([1, ST], f32, tag="spp", name=f"spp{b}_{st}")
            tts = [None] * UC

            def matvec(j):
                vj = v_all[:, j:j + 1].bitcast(f32)
                if j == 0:
                    nc.vector.tensor_scalar_mul(
                        out=acc[:], in0=tts[j][:].bitcast(f32), scalar1=vj
                    )
                else:
                    nc.vector.tensor_scalar_mul(
                        out=tmp[:], in0=tts[j][:].bitcast(f32), scalar1=vj
                    )
                    nc.vector.tensor_tensor(
                        out=acc[:], in0=acc[:].bitcast(f32), in1=tmp[:],
                        op=mybir.AluOpType.add,
                    )

            for j in range(UC):
                if pre_j is not None:
                    pre_j(j)
                pj = pp.tile([P, ST], f32, tag="proj", name=f"pj{b}_{st}_{j}")
                for k in range(KC):
                    nc.tensor.matmul(
                        pj[:],
                        w1_lhsT(k, j),
                        xts[:, k * ST:(k + 1) * ST],
                        start=(k == 0),
                        stop=(k == KC - 1),
                    )
                tts[j] = ttp.tile([P, ST], f32r, tag="tt", name=f"tt{b}_{st}_{j}")
                nc.scalar.activation(
                    tts[j][:], pj[:], AF.Tanh, bias=bias_sb[j][:, b:b + 1]
                )
                if j >= 1:
                    matvec(j - 1)
            matvec(UC - 1)
            nc.tensor.matmul(spp[:], one_sb[:], acc[:], start=True, stop=True)
            return spp

        def exp_scatter(b, st, spp, lo=0, hi=ST):
            """exp (+partial sum) of score columns [lo, hi); scatter so that
            pcol[p, t] = piece[p*tp + t] (partition-major within the piece)."""
            pr, zp, _ = batch_state(b)
            n = hi - lo
            tp = n // P
            zslot = st if lo == 0 else NST + st % NST
            nc.scalar.activation(
                pr[:, st * ST + lo:st * ST + hi],
                spp[:] if (lo == 0 and hi == ST) else spp[:, 0:n],
                AF.Exp,
                accum_out=zp[:, zslot:zslot + 1],
            )
            pbt = dramp.tile([1, n], f32, tag="pb", name=f"pb{b}_{st}_{lo}")
            nc.gpsimd.dma_start(out=pbt[:], in_=pr[:, st * ST + lo:st * ST + hi])
            pcol = smallp.tile([P, tp], f32r, tag="pcol", name=f"pc{b}_{st}_{lo}")
            nc.gpsimd.dma_start(
                out=pcol[:],
                in_=pbt[:].rearrange("a (p t) -> p (a t)", p=P).bitcast(f32r),
            )
            return pcol

        def ctx_mms(b, st, pcol, tp=TPT, piece_lo=None, first=False, last=False):
            """context matmuls (unnormalized weights) for one s-tile or piece."""
            _, _, cps = batch_state(b)
            if piece_lo is None:
                # rows follow the whole-tile mapping s = st*ST + p*TPT + t
                nv = nvp.tile([P, TPT * D], f32r, tag="nv", name=f"nv{b}_{st}")
                nc.sync.dma_start(
                    out=nv[:],
                    in_=val[b]
                    .rearrange("(g p t) d -> g p (t d)", p=P, t=TPT)[st]
                    .bitcast(f32r),
                )
            else:
                # piece mapping: rows s = st*ST + piece_lo + p*tp + t
                nv = nvp.tile([P, tp * D], f32r, tag="nv", name=f"nv{b}_{st}_{piece_lo}")
                nc.sync.dma_start(
                    out=nv[:],
                    in_=val[b, st * ST + piece_lo:st * ST + piece_lo + tp * P, :]
                    .rearrange("(p t) d -> p (t d)", p=P)
                    .bitcast(f32r),
                )
            for tloc in range(tp):
                for dn in range(2):
                    nc.tensor.matmul(
                        cps[dn][:],
                        pcol[:, tloc:tloc + 1],
                        nv[:, tloc * D + dn * ST:tloc * D + (dn + 1) * ST],
                        start=(first and tloc == 0),
                        stop=(last and tloc == tp - 1),
                    )

        def finalize(b):
            """1/Z normalization of both outputs; DMA out."""
            pr, zp, cps = batch_state(b)
            z = smallp.tile([1, 1], f32, tag="z", name=f"z{b}")
            nc.vector.reduce_sum(out=z[:], in_=zp[:], axis=AX.X)
            rz = smallp.tile([1, 1], f32, tag="rz", name=f"rz{b}")
            nc.vector.reciprocal(rz[:], z[:])
            at = rowp.tile([1, S], f32, tag="prow", name=f"at{b}")
            nc.vector.tensor_scalar_mul(out=at[:], in0=pr[:], scalar1=rz[:, 0:1])
            nc.gpsimd.dma_start(out=oattn[b:b + 1, :], in_=at[:])
            crow = smallp.tile([1, D], f32, tag="crow", name=f"cr{b}", bufs=2)
            for dn in range(2):
                nc.vector.tensor_scalar_mul(
                    out=crow[:, dn * ST:(dn + 1) * ST], in0=cps[dn][:], scalar1=rz[:, 0:1]
                )
            nc.gpsimd.dma_start(out=octx[b:b + 1, :], in_=crow[:])
            del state[b]

        # s-tile software pipeline: ctx matmuls of tile i run after the score
        # matmuls of tile i+1, so the PE never waits on exp/scatter.
        tasks = [(b, st) for b in range(NB) for st in range(NST)]
        pend = None
        for idx, (b, st) in enumerate(tasks):
            last = idx == len(tasks) - 1
            if b == 0 and st == 0:
                spp = score_stile(
                    b, st, xts=first_xts,
                    pre_j=lambda j: (proj_q(j) if j >= UC // 2 else None),
                )
            else:
                spp = score_stile(b, st)
            if last:
                # emit the final exp/scatter before pend's nv DMAs so the
                # tail-critical scatter isn't queued behind bulk traffic
                mine = exp_scatter(b, st, spp)
            if pend is not None:
                pb_, pst_, pcol_ = pend
                ctx_mms(pb_, pst_, pcol_, first=(pst_ == 0),
                        last=(pst_ == NST - 1))
                if pst_ == NST - 1:
                    finalize(pb_)
            pend = (b, st, exp_scatter(b, st, spp)) if not last else (b, st, mine)
        pb_, pst_, pcol_ = pend
        # keep the PE busy (and the HAM clock warm) while the final
        # exp/scatter chain completes
        tdps = pp.tile([P, ST], f32, tag="proj", name="tdps")
        for i in range(12):
            nc.tensor.matmul(tdps[:], zt[:, 0:P], zt[:], start=True, stop=True)
        ctx_mms(pb_, pst_, pcol_, first=(pst_ == 0), last=(pst_ == NST - 1))
        finalize(pb_)

    nc.compile()
    return nc


def kernel(query, values, W1, b1, W2, b2, V, bV, _trace=False, _trace_kwargs=None):
    from concourse.bass_utils import run_bass_kernel_spmd

    query = np.asarray(query, dtype=np.float32)
    values = np.asarray(values, dtype=np.float32)
    W1 = np.asarray(W1, dtype=np.float32)
    b1 = np.asarray(b1, dtype=np.float32)
    W2 = np.asarray(W2, dtype=np.float32)
    b2 = np.asarray(b2, dtype=np.float32)
    V = np.asarray(V, dtype=np.float32)

    assert query.shape == (B, D) and values.shape == (B, S, D)

    if "nc" not in _CACHE:
        _CACHE["nc"] = _build()
    nc = _CACHE["nc"]

    valuesT = np.ascontiguousarray(values.transpose(0, 2, 1))  # [B, D, S]
    qTf = np.ascontiguousarray(query.T)                        # [D, B]
    bcf = np.ascontiguousarray((b1 + b2).reshape(U, 1))
    Vf = np.ascontiguousarray(V.reshape(U, 1))

    in_maps = []
    for c in range(NC):
        lo, hi = c * NB, (c + 1) * NB
        in_maps.append({
            "xt": valuesT[lo:hi],
            "val": values[lo:hi],
            "qT": np.ascontiguousarray(qTf[:, lo:hi]),
            "w1": W1,
            "w2": W2,
            "bc": bcf,
            "vv": Vf,
            "one": np.ones((P, 1), np.float32),
        })

    try:
        res = run_bass_kernel_spmd(
            nc, in_maps, list(range(NC)), trace=_trace, **(_trace_kwargs or {})
        )
    except Exception:
        # transient device/axon flake: retry once
        res = run_bass_kernel_spmd(
            nc, in_maps, list(range(NC)), trace=_trace, **(_trace_kwargs or {})
        )
    _CACHE["last_result"] = res

    context = np.concatenate([res.results[c]["octx"] for c in range(NC)], axis=0)
    attn = np.concatenate([res.results[c]["oattn"] for c in range(NC)], axis=0)
    return context, attn.reshape(B, S, 1)
